# revision 24
# baseline (speedup 1.0000x reference)
"""AdaptiveGraphConv Trainium2 kernel, data-parallel over batch on 8 NeuronCores.

Reference computation (per full input):
  sim  = relu(E @ E^T)                               [N, N]
  d[n] = 1 + softmax(sim, axis=1)[n, n]              (diag gate)
  Ew   = einsum('nd,dcf->ncf', E, W)                 per-node weights
  eb   = E @ bias                                    per-node bias [N, F]
  y[b,t,n,f] = (d[n] * x[b,t,n,:]) @ Ew[n] + eb[n]

Device strategy per core (2 of 16 batches, R = 2*288 = 576 rows):
  - host supplies x transposed to node-major [N, C, R] so each node-pair is a
    contiguous [128, 576] block (C on partitions = matmul contraction layout)
  - on-chip: compute d, fold it into E (E' = diag(d) @ E), build block-diagonal
    per-pair stationary weights Ew (fp32r), then stream 104 node-pairs:
    2 fp32r matmuls per pair (row halves), fused bias-add written in place
    over the x tile (DVE, f32r). Loads and stores ride ONE HWDGE ring in
    44-pair bursts so HBM traffic stays unidirectional per burst (mixed
    read/write measurably collapses DMA throughput ~2x on this part).
  - host un-permutes y^T shards back to [B, T, N, F].
"""

import sys

sys.path.insert(0, "/opt/trn_rl_repo")

from contextlib import ExitStack

import numpy as np

N_CORES = 8
NODE = 207
NODE_P = 208  # padded to even node count
PAIRS = NODE_P // 2  # 104
EMB = 128
C = 64
F = 64
B = 16
T = 288
B_SH = B // N_CORES  # 2
R = B_SH * T  # 576 rows per core
RH = R // 2  # 288, matmul free-dim chunk

_CACHE = {}


def _build(repeat=1, nb=1, bufs=44, mix=3, ph=44):
    import concourse.tile as tile
    from concourse import bacc, mybir

    f32 = mybir.dt.float32
    f32r = mybir.dt.float32r
    AF = mybir.ActivationFunctionType
    ALU = mybir.AluOpType
    AX = mybir.AxisListType

    nc = bacc.Bacc("TRN2", target_bir_lowering=False, debug=False, num_devices=N_CORES)
    xt = nc.dram_tensor("xt", [NODE_P * C, R], f32r, kind="ExternalInput").ap()
    emb = nc.dram_tensor("emb", [NODE_P, EMB], f32, kind="ExternalInput").ap()
    w = nc.dram_tensor("w", [EMB, C * F], f32, kind="ExternalInput").ap()
    bias_d = nc.dram_tensor("bias", [EMB, F], f32, kind="ExternalInput").ap()
    eye = nc.dram_tensor("eye", [NODE_P, NODE], f32, kind="ExternalInput").ap()
    ident = nc.dram_tensor("ident", [128, 128], f32, kind="ExternalInput").ap()
    yt = nc.dram_tensor("yt", [NODE_P * C, R], f32r, kind="ExternalOutput").ap()

    with tile.TileContext(nc) as tc, ExitStack() as ctx:
        const_pool = ctx.enter_context(tc.tile_pool(name="const", bufs=1))
        small_pool = ctx.enter_context(tc.tile_pool(name="small", bufs=1))
        psum_prep = ctx.enter_context(tc.tile_pool(name="pprep", bufs=2, space="PSUM"))
        psum_main = ctx.enter_context(tc.tile_pool(name="pmain", bufs=6, space="PSUM"))
        xpool = ctx.enter_context(tc.tile_pool(name="xin", bufs=bufs))
        opool = ctx.enter_context(tc.tile_pool(name="yout", bufs=bufs))

        # ---- small constant loads
        ident_sb = const_pool.tile([128, 128], f32)
        nc.sync.dma_start(ident_sb[:], ident[:])
        e1 = const_pool.tile([128, EMB], f32)
        nc.sync.dma_start(e1[:], emb[0:128, :])
        e2 = const_pool.tile([80, EMB], f32, tag="e2")
        nc.sync.dma_start(e2[:], emb[128:NODE_P, :])
        bias_sb = const_pool.tile([128, F], f32)
        nc.sync.dma_start(bias_sb[:], bias_d[:])
        eye1 = const_pool.tile([128, NODE], f32)
        nc.sync.dma_start(eye1[:], eye[0:128, :])
        eye2 = const_pool.tile([80, NODE], f32, tag="eye2")
        nc.sync.dma_start(eye2[:], eye[128:NODE_P, :])

        # ---- E^T (unscaled) via PE transpose
        ET = small_pool.tile([128, NODE_P], f32)
        tp1 = psum_prep.tile([128, 128], f32, tag="prep")
        nc.tensor.transpose(tp1[:], e1[:], ident_sb[:])
        nc.vector.tensor_copy(ET[:, 0:128], tp1[:])
        tp2 = psum_prep.tile([128, 80], f32, tag="prep")
        nc.tensor.transpose(tp2[:], e2[:], ident_sb[0:80, 0:80])
        nc.vector.tensor_copy(ET[:, 128:NODE_P], tp2[:])

        # ---- sim = relu(E E^T) per row-tile; d = 1 + softmax diag
        def diag_gate(rows, off, e_tile, eye_tile):
            simp_t = psum_prep.tile([128, NODE_P], f32, tag="prep")
            simp = simp_t[0:rows, :]
            nc.tensor.matmul(simp, ET[:, off : off + rows], ET[:])
            s_t = small_pool.tile([128, NODE_P], f32, tag=f"s{off}")
            s = s_t[0:rows, :]
            nc.vector.tensor_relu(s[:], simp[:])
            m_t = small_pool.tile([128, 1], f32, tag=f"m{off}")
            m = m_t[0:rows, :]
            nc.vector.tensor_reduce(m[:], s[:, 0:NODE], AX.X, ALU.max)
            negm_t = small_pool.tile([128, 1], f32, tag=f"negm{off}")
            negm = negm_t[0:rows, :]
            nc.vector.tensor_scalar_mul(negm[:], m[:], -1.0)
            ex_t = small_pool.tile([128, NODE], f32, tag=f"ex{off}")
            ex = ex_t[0:rows, :]
            nc.scalar.activation(ex[:], s[:, 0:NODE], AF.Exp, bias=negm[:])
            z_t = small_pool.tile([128, 1], f32, tag=f"z{off}")
            z = z_t[0:rows, :]
            nc.vector.tensor_reduce(z[:], ex[:], AX.X, ALU.add)
            msk_t = small_pool.tile([128, NODE], f32, tag=f"msk{off}")
            msk = msk_t[0:rows, :]
            nc.vector.tensor_mul(msk[:], s[:, 0:NODE], eye_tile[:])
            dg_t = small_pool.tile([128, 1], f32, tag=f"dg{off}")
            dg = dg_t[0:rows, :]
            nc.vector.tensor_reduce(dg[:], msk[:], AX.X, ALU.add)
            ed_t = small_pool.tile([128, 1], f32, tag=f"ed{off}")
            ed = ed_t[0:rows, :]
            nc.scalar.activation(ed[:], dg[:], AF.Exp, bias=negm[:])
            rz_t = small_pool.tile([128, 1], f32, tag=f"rz{off}")
            rz = rz_t[0:rows, :]
            nc.vector.reciprocal(rz[:], z[:])
            d_t = small_pool.tile([128, 1], f32, tag=f"d{off}")
            d = d_t[0:rows, :]
            nc.vector.tensor_mul(d[:], ed[:], rz[:])
            nc.vector.tensor_scalar_add(d[:], d[:], 1.0)
            # E' = diag(d) @ E rows
            ep_t = small_pool.tile([128, EMB], f32, tag=f"ep{off}")
            ep = ep_t[0:rows, :]
            nc.vector.tensor_scalar_mul(ep[:], e_tile[:], d[:])
            return ep

        ep1 = diag_gate(128, 0, e1, eye1)
        ep2 = diag_gate(80, 128, e2, eye2)

        # ---- E'^T via PE transpose
        EpT = small_pool.tile([128, NODE_P], f32)
        tq1 = psum_prep.tile([128, 128], f32, tag="prep")
        nc.tensor.transpose(tq1[:], ep1[:], ident_sb[:])
        nc.vector.tensor_copy(EpT[:, 0:128], tq1[:])
        tq2 = psum_prep.tile([128, 80], f32, tag="prep")
        nc.tensor.transpose(tq2[:], ep2[:], ident_sb[0:80, 0:80])
        nc.vector.tensor_copy(EpT[:, 128:NODE_P], tq2[:])

        # ---- per-node bias, pair-stacked: ebT2[64*par + f, p] = eb[2p+par, f]
        ebT2 = const_pool.tile([128, PAIRS], f32)
        pe = psum_prep.tile([128, PAIRS], f32, tag="prep")
        nc.tensor.matmul(pe[0:64, :], bias_sb[:], ET[:, 0:NODE_P:2])
        nc.tensor.matmul(pe[64:128, :], bias_sb[:], ET[:, 1:NODE_P:2])
        nc.vector.tensor_copy(ebT2[:], pe[:])

        # ---- stationary weights, block-diagonal per pair:
        #   Ew[c,       p*128 + f]      = sum_d E'[2p,   d] W[d, c, f]
        #   Ew[64 + c,  p*128 + 64 + f] = sum_d E'[2p+1, d] W[d, c, f]
        # (off-diagonal quadrants stay zero)
        Ew = const_pool.tile([128, 128 * PAIRS], f32r)
        # memset can't produce f32r-typed output; zero via converting copies
        zsrc = small_pool.tile([128, 8 * PAIRS], f32)
        nc.vector.memset(zsrc[:], 0.0)
        Ew3 = Ew[:].rearrange("p (q b) -> p q b", b=128)
        zs3 = zsrc[:].rearrange("p (q b) -> p q b", b=8)
        for s in range(8):
            nc.vector.tensor_copy(Ew3[0:64, :, 64 + 8 * s : 72 + 8 * s], zs3[0:64, :, :])
            nc.vector.tensor_copy(Ew3[64:128, :, 8 * s : 8 * (s + 1)], zs3[64:128, :, :])
        wpool = ctx.enter_context(tc.tile_pool(name="wsl", bufs=4))
        for f in range(F):
            pf = psum_prep.tile([128, PAIRS], f32, tag="prep")
            wsl = wpool.tile([128, C], f32)
            nc.sync.dma_start(wsl[:], w[:, f * C : (f + 1) * C])
            wf = wsl[:]  # W[:, :, f] -> [128, 64] (w is f-major from host)
            nc.tensor.matmul(pf[0:64, :], wf, EpT[:, 0:NODE_P:2])
            nc.tensor.matmul(pf[64:128, :], wf, EpT[:, 1:NODE_P:2])
            nc.vector.tensor_copy(Ew[0:64, f :: 128], pf[0:64, :])
            nc.vector.tensor_copy(Ew[64:128, (64 + f) :: 128], pf[64:128, :])

        # ---- main streaming loop over node pairs, nb pairs per DMA transfer
        xt3 = xt.rearrange("(p q) r -> p q r", q=128)
        yt3 = yt.rearrange("(p q) r -> p q r", q=128)

        def pair_compute(p, x2):
            # 2 blockdiag matmuls + bias-add written back IN PLACE over x2
            # (f32r out: DVE rounds, keeps the verifier's f32r typing happy)
            ew_p = Ew[:, p * 128 : (p + 1) * 128]
            for h in range(2):
                ps = psum_main.tile([128, RH], f32)
                cols = slice(h * RH, (h + 1) * RH)
                nc.tensor.matmul(ps[:], ew_p, x2[:, cols])
                nc.vector.tensor_scalar_add(x2[:, cols], ps[:], ebT2[:, p : p + 1])
            return x2

        def main_loop():
            if mix == 3:
                # burst-phased on ONE HWDGE ring: the ring's FIFO keeps the
                # load burst and store burst of each phase apart, so HBM
                # traffic stays unidirectional per burst (mixed read/write
                # collapses DMA throughput ~2x, measured)
                for k0 in range(0, PAIRS, ph):
                    phase = []
                    for p in range(k0, min(k0 + ph, PAIRS)):
                        x2 = xpool.tile([128, R], f32r)
                        nc.sync.dma_start(x2[:], xt[p * 128 : (p + 1) * 128, :])
                        phase.append((p, x2))
                    for p, x2 in phase:
                        pair_compute(p, x2)
                        nc.sync.dma_start(yt[p * 128 : (p + 1) * 128, :], x2[:])
                return
            for i0, p0 in enumerate(range(0, PAIRS, nb)):
                if mix == 1:
                    ld_eng = nc.sync if i0 % 2 == 0 else nc.scalar
                    st_eng = nc.scalar if i0 % 2 == 0 else nc.sync
                elif mix == 2:
                    ld_eng, st_eng = nc.sync, nc.gpsimd
                else:
                    ld_eng, st_eng = nc.sync, nc.scalar
                x2 = xpool.tile([128, nb * R], f32r)
                ld_eng.dma_start(
                    x2[:].rearrange("q (b r) -> q b r", b=nb),
                    xt3[p0 : p0 + nb, :, :].rearrange("b q r -> q b r"),
                )
                stage = opool.tile([128, nb * R], f32r)
                for j in range(nb):
                    p = p0 + j
                    ew_p = Ew[:, p * 128 : (p + 1) * 128]
                    for h in range(2):
                        ps = psum_main.tile([128, RH], f32)
                        cols = slice(j * R + h * RH, j * R + (h + 1) * RH)
                        nc.tensor.matmul(ps[:], ew_p, x2[:, cols])
                        nc.vector.tensor_scalar_add(
                            stage[:, cols], ps[:], ebT2[:, p : p + 1]
                        )
                st_eng.dma_start(
                    yt3[p0 : p0 + nb, :, :].rearrange("b q r -> q b r"),
                    stage[:].rearrange("q (b r) -> q b r", b=nb),
                )

        if repeat == 1:
            main_loop()
        else:
            # hardware loop: one NEFF execution = `repeat` full streaming passes
            # (benchmarking only; kernel() uses repeat=1)
            with tc.For_i(0, repeat, 1):
                main_loop()

    nc.compile()
    return nc


def _get_nc(repeat=1, nb=1, bufs=44, mix=3, ph=44):
    key = f"nc{repeat}_{nb}_{bufs}_{mix}_{ph}"
    if key not in _CACHE:
        _CACHE[key] = _build(repeat, nb, bufs, mix, ph)
    return _CACHE[key]


def kernel(x, node_embedding, weights, bias):
    from concourse.bass_utils import run_bass_kernel_spmd

    nc = _get_nc()

    emb_p = np.zeros((NODE_P, EMB), np.float32)
    emb_p[:NODE] = node_embedding
    w2 = np.ascontiguousarray(weights.transpose(0, 2, 1).reshape(EMB, F * C), np.float32)
    bias_f = np.ascontiguousarray(bias, np.float32)
    eye_np = np.eye(NODE_P, NODE, dtype=np.float32)
    ident_np = np.eye(128, dtype=np.float32)

    in_maps = []
    for i in range(N_CORES):
        xi = x[B_SH * i : B_SH * (i + 1)]  # [2, T, NODE, C]
        xt = np.zeros((NODE_P, C, R), np.float32)
        xt[:NODE] = np.asarray(xi).transpose(2, 3, 0, 1).reshape(NODE, C, R)
        in_maps.append(
            {
                "xt": xt.reshape(NODE_P * C, R),
                "emb": emb_p,
                "w": w2,
                "bias": bias_f,
                "eye": eye_np,
                "ident": ident_np,
            }
        )

    res = run_bass_kernel_spmd(nc, in_maps, core_ids=list(range(N_CORES)))

    out = np.empty((B, T, NODE, F), np.float32)
    for i in range(N_CORES):
        ytr = res.results[i]["yt"].reshape(PAIRS, 2, F, B_SH, T)
        y_local = ytr.transpose(3, 4, 0, 1, 2).reshape(B_SH, T, NODE_P, F)
        out[B_SH * i : B_SH * (i + 1)] = y_local[:, :, :NODE, :]
    return out


# revision 25
# speedup vs baseline: 1.2382x; 1.2382x over previous
"""AdaptiveGraphConv Trainium2 kernel, data-parallel over batch on 8 NeuronCores.

Reference computation (per full input):
  sim  = relu(E @ E^T)                               [N, N]
  d[n] = 1 + softmax(sim, axis=1)[n, n]              (diag gate)
  Ew   = einsum('nd,dcf->ncf', E, W)                 per-node weights
  eb   = E @ bias                                    per-node bias [N, F]
  y[b,t,n,f] = (d[n] * x[b,t,n,:]) @ Ew[n] + eb[n]

Device strategy per core (2 of 16 batches, R = 2*288 = 576 rows):
  - host supplies x transposed to node-major [N, C, R] so each node-pair is a
    contiguous [128, 576] block (C on partitions = matmul contraction layout)
  - on-chip: compute d, fold it into E (E' = diag(d) @ E), build block-diagonal
    per-pair stationary weights Ew (fp32r), then stream 104 node-pairs:
    2 fp32r matmuls per pair (row halves), fused bias-add written in place
    over the x tile (DVE, f32r). Loads and stores ride ONE HWDGE ring in
    44-pair bursts so HBM traffic stays unidirectional per burst (mixed
    read/write measurably collapses DMA throughput ~2x on this part).
  - host un-permutes y^T shards back to [B, T, N, F].
"""

import sys

sys.path.insert(0, "/opt/trn_rl_repo")

from contextlib import ExitStack

import numpy as np

N_CORES = 8
NODE = 207
NODE_P = 208  # padded to even node count
PAIRS = NODE_P // 2  # 104
EMB = 128
C = 64
F = 64
B = 16
T = 288
B_SH = B // N_CORES  # 2
R = B_SH * T  # 576 rows per core
RH = R // 2  # 288, matmul free-dim chunk

_CACHE = {}


def _build(repeat=1, nb=1, bufs=44, mix=3, ph=44):
    import concourse.tile as tile
    from concourse import bacc, mybir

    f32 = mybir.dt.float32
    f32r = mybir.dt.float32r
    AF = mybir.ActivationFunctionType
    ALU = mybir.AluOpType
    AX = mybir.AxisListType

    nc = bacc.Bacc("TRN2", target_bir_lowering=False, debug=False, num_devices=N_CORES)
    xt = nc.dram_tensor("xt", [NODE_P * C, R], f32r, kind="ExternalInput").ap()
    emb = nc.dram_tensor("emb", [NODE_P, EMB], f32, kind="ExternalInput").ap()
    w = nc.dram_tensor("w", [EMB, C * F], f32, kind="ExternalInput").ap()
    bias_d = nc.dram_tensor("bias", [EMB, F], f32, kind="ExternalInput").ap()
    eye = nc.dram_tensor("eye", [NODE_P, NODE], f32, kind="ExternalInput").ap()
    ident = nc.dram_tensor("ident", [128, 128], f32, kind="ExternalInput").ap()
    yt = nc.dram_tensor("yt", [NODE_P * C, R], f32r, kind="ExternalOutput").ap()

    with tile.TileContext(nc) as tc, ExitStack() as ctx:
        const_pool = ctx.enter_context(tc.tile_pool(name="const", bufs=1))
        small_pool = ctx.enter_context(tc.tile_pool(name="small", bufs=1))
        psum_prep = ctx.enter_context(tc.tile_pool(name="pprep", bufs=2, space="PSUM"))
        psum_main = ctx.enter_context(tc.tile_pool(name="pmain", bufs=6, space="PSUM"))
        xpool = ctx.enter_context(tc.tile_pool(name="xin", bufs=bufs))
        opool = ctx.enter_context(tc.tile_pool(name="yout", bufs=bufs))

        # ---- small constant loads
        ident_sb = const_pool.tile([128, 128], f32)
        nc.sync.dma_start(ident_sb[:], ident[:])
        e1 = const_pool.tile([128, EMB], f32)
        nc.sync.dma_start(e1[:], emb[0:128, :])
        e2 = const_pool.tile([80, EMB], f32, tag="e2")
        nc.sync.dma_start(e2[:], emb[128:NODE_P, :])
        bias_sb = const_pool.tile([128, F], f32)
        nc.sync.dma_start(bias_sb[:], bias_d[:])
        eye1 = const_pool.tile([128, NODE], f32)
        nc.sync.dma_start(eye1[:], eye[0:128, :])
        eye2 = const_pool.tile([80, NODE], f32, tag="eye2")
        nc.sync.dma_start(eye2[:], eye[128:NODE_P, :])

        # ---- E^T (unscaled) via PE transpose
        ET = small_pool.tile([128, NODE_P], f32)
        tp1 = psum_prep.tile([128, 128], f32, tag="prep")
        nc.tensor.transpose(tp1[:], e1[:], ident_sb[:])
        nc.vector.tensor_copy(ET[:, 0:128], tp1[:])
        tp2 = psum_prep.tile([128, 80], f32, tag="prep")
        nc.tensor.transpose(tp2[:], e2[:], ident_sb[0:80, 0:80])
        nc.vector.tensor_copy(ET[:, 128:NODE_P], tp2[:])

        # ---- sim = relu(E E^T) per row-tile; d = 1 + softmax diag
        def diag_gate(rows, off, e_tile, eye_tile):
            simp_t = psum_prep.tile([128, NODE_P], f32, tag="prep")
            simp = simp_t[0:rows, :]
            nc.tensor.matmul(simp, ET[:, off : off + rows], ET[:])
            s_t = small_pool.tile([128, NODE_P], f32, tag=f"s{off}")
            s = s_t[0:rows, :]
            nc.vector.tensor_relu(s[:], simp[:])
            m_t = small_pool.tile([128, 1], f32, tag=f"m{off}")
            m = m_t[0:rows, :]
            nc.vector.tensor_reduce(m[:], s[:, 0:NODE], AX.X, ALU.max)
            negm_t = small_pool.tile([128, 1], f32, tag=f"negm{off}")
            negm = negm_t[0:rows, :]
            nc.vector.tensor_scalar_mul(negm[:], m[:], -1.0)
            ex_t = small_pool.tile([128, NODE], f32, tag=f"ex{off}")
            ex = ex_t[0:rows, :]
            nc.scalar.activation(ex[:], s[:, 0:NODE], AF.Exp, bias=negm[:])
            z_t = small_pool.tile([128, 1], f32, tag=f"z{off}")
            z = z_t[0:rows, :]
            nc.vector.tensor_reduce(z[:], ex[:], AX.X, ALU.add)
            msk_t = small_pool.tile([128, NODE], f32, tag=f"msk{off}")
            msk = msk_t[0:rows, :]
            nc.vector.tensor_mul(msk[:], s[:, 0:NODE], eye_tile[:])
            dg_t = small_pool.tile([128, 1], f32, tag=f"dg{off}")
            dg = dg_t[0:rows, :]
            nc.vector.tensor_reduce(dg[:], msk[:], AX.X, ALU.add)
            ed_t = small_pool.tile([128, 1], f32, tag=f"ed{off}")
            ed = ed_t[0:rows, :]
            nc.scalar.activation(ed[:], dg[:], AF.Exp, bias=negm[:])
            rz_t = small_pool.tile([128, 1], f32, tag=f"rz{off}")
            rz = rz_t[0:rows, :]
            nc.vector.reciprocal(rz[:], z[:])
            d_t = small_pool.tile([128, 1], f32, tag=f"d{off}")
            d = d_t[0:rows, :]
            nc.vector.tensor_mul(d[:], ed[:], rz[:])
            nc.vector.tensor_scalar_add(d[:], d[:], 1.0)
            # E' = diag(d) @ E rows
            ep_t = small_pool.tile([128, EMB], f32, tag=f"ep{off}")
            ep = ep_t[0:rows, :]
            nc.vector.tensor_scalar_mul(ep[:], e_tile[:], d[:])
            return ep

        ep1 = diag_gate(128, 0, e1, eye1)
        ep2 = diag_gate(80, 128, e2, eye2)

        # ---- E'^T via PE transpose
        EpT = small_pool.tile([128, NODE_P], f32)
        tq1 = psum_prep.tile([128, 128], f32, tag="prep")
        nc.tensor.transpose(tq1[:], ep1[:], ident_sb[:])
        nc.vector.tensor_copy(EpT[:, 0:128], tq1[:])
        tq2 = psum_prep.tile([128, 80], f32, tag="prep")
        nc.tensor.transpose(tq2[:], ep2[:], ident_sb[0:80, 0:80])
        nc.vector.tensor_copy(EpT[:, 128:NODE_P], tq2[:])

        # ---- per-node bias, pair-stacked: ebT2[64*par + f, p] = eb[2p+par, f]
        ebT2 = const_pool.tile([128, PAIRS], f32)
        pe = psum_prep.tile([128, PAIRS], f32, tag="prep")
        nc.tensor.matmul(pe[0:64, :], bias_sb[:], ET[:, 0:NODE_P:2])
        nc.tensor.matmul(pe[64:128, :], bias_sb[:], ET[:, 1:NODE_P:2])
        nc.vector.tensor_copy(ebT2[:], pe[:])

        # ---- stationary weights, block-diagonal per pair:
        #   Ew[c,       p*128 + f]      = sum_d E'[2p,   d] W[d, c, f]
        #   Ew[64 + c,  p*128 + 64 + f] = sum_d E'[2p+1, d] W[d, c, f]
        # (off-diagonal quadrants stay zero)
        Ew = const_pool.tile([128, 128 * PAIRS], f32r)
        # memset can't produce f32r-typed output; zero via converting copies
        zsrc = small_pool.tile([128, 8 * PAIRS], f32)
        nc.vector.memset(zsrc[:], 0.0)
        Ew3 = Ew[:].rearrange("p (q b) -> p q b", b=128)
        zs3 = zsrc[:].rearrange("p (q b) -> p q b", b=8)
        for s in range(8):
            nc.vector.tensor_copy(Ew3[0:64, :, 64 + 8 * s : 72 + 8 * s], zs3[0:64, :, :])
            nc.vector.tensor_copy(Ew3[64:128, :, 8 * s : 8 * (s + 1)], zs3[64:128, :, :])
        wpool = ctx.enter_context(tc.tile_pool(name="wsl", bufs=4))
        for f in range(F):
            pf = psum_prep.tile([128, PAIRS], f32, tag="prep")
            wsl = wpool.tile([128, C], f32)
            nc.sync.dma_start(wsl[:], w[:, f * C : (f + 1) * C])
            wf = wsl[:]  # W[:, :, f] -> [128, 64] (w is f-major from host)
            nc.tensor.matmul(pf[0:64, :], wf, EpT[:, 0:NODE_P:2])
            nc.tensor.matmul(pf[64:128, :], wf, EpT[:, 1:NODE_P:2])
            nc.vector.tensor_copy(Ew[0:64, f :: 128], pf[0:64, :])
            nc.vector.tensor_copy(Ew[64:128, (64 + f) :: 128], pf[64:128, :])

        # ---- main streaming loop over node pairs, nb pairs per DMA transfer
        xt3 = xt.rearrange("(p q) r -> p q r", q=128)
        yt3 = yt.rearrange("(p q) r -> p q r", q=128)

        def pair_compute(p, x2):
            # 2 blockdiag matmuls + bias-add written back IN PLACE over x2
            # (f32r out: DVE rounds, keeps the verifier's f32r typing happy)
            ew_p = Ew[:, p * 128 : (p + 1) * 128]
            for h in range(2):
                ps = psum_main.tile([128, RH], f32)
                cols = slice(h * RH, (h + 1) * RH)
                nc.tensor.matmul(ps[:], ew_p, x2[:, cols])
                nc.vector.tensor_scalar_add(x2[:, cols], ps[:], ebT2[:, p : p + 1])
            return x2

        def main_loop():
            if mix == 3:
                # burst-phased on ONE HWDGE ring: the ring's FIFO keeps the
                # load burst and store burst of each phase apart, so HBM
                # traffic stays unidirectional per burst (mixed read/write
                # collapses DMA throughput ~2x, measured); nb pairs per DMA
                for k0 in range(0, PAIRS, ph):
                    phase = []
                    for p0 in range(k0, min(k0 + ph, PAIRS), nb):
                        x2 = xpool.tile([128, nb * R], f32r)
                        if nb == 1:
                            nc.sync.dma_start(x2[:], xt[p0 * 128 : (p0 + 1) * 128, :])
                        else:
                            nc.sync.dma_start(
                                x2[:].rearrange("q (b r) -> q b r", b=nb),
                                xt3[p0 : p0 + nb, :, :].rearrange("b q r -> q b r"),
                            )
                        phase.append((p0, x2))
                    for p0, x2 in phase:
                        for j in range(nb):
                            p = p0 + j
                            ew_p = Ew[:, p * 128 : (p + 1) * 128]
                            for h in range(2):
                                ps = psum_main.tile([128, RH], f32)
                                cols = slice(j * R + h * RH, j * R + (h + 1) * RH)
                                nc.tensor.matmul(ps[:], ew_p, x2[:, cols])
                                nc.vector.tensor_scalar_add(
                                    x2[:, cols], ps[:], ebT2[:, p : p + 1]
                                )
                        if nb == 1:
                            nc.sync.dma_start(yt[p0 * 128 : (p0 + 1) * 128, :], x2[:])
                        else:
                            nc.sync.dma_start(
                                yt3[p0 : p0 + nb, :, :].rearrange("b q r -> q b r"),
                                x2[:].rearrange("q (b r) -> q b r", b=nb),
                            )
                return
            for i0, p0 in enumerate(range(0, PAIRS, nb)):
                if mix == 1:
                    ld_eng = nc.sync if i0 % 2 == 0 else nc.scalar
                    st_eng = nc.scalar if i0 % 2 == 0 else nc.sync
                elif mix == 2:
                    ld_eng, st_eng = nc.sync, nc.gpsimd
                else:
                    ld_eng, st_eng = nc.sync, nc.scalar
                x2 = xpool.tile([128, nb * R], f32r)
                ld_eng.dma_start(
                    x2[:].rearrange("q (b r) -> q b r", b=nb),
                    xt3[p0 : p0 + nb, :, :].rearrange("b q r -> q b r"),
                )
                stage = opool.tile([128, nb * R], f32r)
                for j in range(nb):
                    p = p0 + j
                    ew_p = Ew[:, p * 128 : (p + 1) * 128]
                    for h in range(2):
                        ps = psum_main.tile([128, RH], f32)
                        cols = slice(j * R + h * RH, j * R + (h + 1) * RH)
                        nc.tensor.matmul(ps[:], ew_p, x2[:, cols])
                        nc.vector.tensor_scalar_add(
                            stage[:, cols], ps[:], ebT2[:, p : p + 1]
                        )
                st_eng.dma_start(
                    yt3[p0 : p0 + nb, :, :].rearrange("b q r -> q b r"),
                    stage[:].rearrange("q (b r) -> q b r", b=nb),
                )

        if repeat == 1:
            main_loop()
        else:
            # hardware loop: one NEFF execution = `repeat` full streaming passes
            # (benchmarking only; kernel() uses repeat=1)
            with tc.For_i(0, repeat, 1):
                main_loop()

    nc.compile()
    return nc


def _get_nc(repeat=1, nb=1, bufs=44, mix=3, ph=44):
    key = f"nc{repeat}_{nb}_{bufs}_{mix}_{ph}"
    if key not in _CACHE:
        _CACHE[key] = _build(repeat, nb, bufs, mix, ph)
    return _CACHE[key]


def kernel(x, node_embedding, weights, bias):
    from concourse.bass_utils import run_bass_kernel_spmd

    nc = _get_nc()

    emb_p = np.zeros((NODE_P, EMB), np.float32)
    emb_p[:NODE] = node_embedding
    w2 = np.ascontiguousarray(weights.transpose(0, 2, 1).reshape(EMB, F * C), np.float32)
    bias_f = np.ascontiguousarray(bias, np.float32)
    eye_np = np.eye(NODE_P, NODE, dtype=np.float32)
    ident_np = np.eye(128, dtype=np.float32)

    in_maps = []
    for i in range(N_CORES):
        xi = x[B_SH * i : B_SH * (i + 1)]  # [2, T, NODE, C]
        xt = np.zeros((NODE_P, C, R), np.float32)
        xt[:NODE] = np.asarray(xi).transpose(2, 3, 0, 1).reshape(NODE, C, R)
        in_maps.append(
            {
                "xt": xt.reshape(NODE_P * C, R),
                "emb": emb_p,
                "w": w2,
                "bias": bias_f,
                "eye": eye_np,
                "ident": ident_np,
            }
        )

    res = run_bass_kernel_spmd(nc, in_maps, core_ids=list(range(N_CORES)))

    out = np.empty((B, T, NODE, F), np.float32)
    for i in range(N_CORES):
        ytr = res.results[i]["yt"].reshape(PAIRS, 2, F, B_SH, T)
        y_local = ytr.transpose(3, 4, 0, 1, 2).reshape(B_SH, T, NODE_P, F)
        out[B_SH * i : B_SH * (i + 1)] = y_local[:, :, :NODE, :]
    return out
